# revision 27
# baseline (speedup 1.0000x reference)
"""Trainium2 Bass kernel for the BoltzAffinityHead replica.

Sequence-parallel over query rows: each of the 8 cores gets LC/8 = 128
query rows of z / dist_bins / priors / pd_mask; s_proxy and params are
replicated. Each core emits its attn_w shard plus a partial masked-pooled
sum; the tiny output MLP runs on the host after summing per-core partials
(the all-reduce step of the sharding hint).

Per-core pipeline:
  * q/k/v projections from LN(s_proxy); biases folded in via an extra
    contraction row, 1/sqrt(dh) folded into w_q, and the constant part of
    the pair-bias LayerNorm folded in via a 65th contraction row of Q/K.
  * z streamed as bf16 [q,(k,c)] tiles; bn_stats gives LN mean/var; PE
    transposes make z^T slices which are then used as the *stationary*
    matmul operand against g*w_zb so zW lands directly in [q,(k,h)]
    layout; dist_bins handled the same way with block-diagonal weights
    (two k per matmul).
  * bias map B[q,k,h] = (zW - mean*A')*rsqrt(var+eps) + dW + prior+mask,
    built with full-rate DVE map ops; scores = QK + B in PSUM; exp on ACT
    with fused denominator accumulation (max-free softmax - logits are
    bounded by construction); per-head interleave into the output tile
    with the 1/Z scale fused.
  * pooled partial: prot-weighted column reduction of exp(scores) on the
    PE, transposed, then a tiny matvec with V.
"""

import numpy as np
import ml_dtypes

import concourse.bass as bass
import concourse.tile as tile
from concourse import bacc, mybir
from concourse.bass_utils import run_bass_kernel_spmd
from concourse.masks import make_identity

FP = mybir.dt.float32
BF = mybir.dt.bfloat16
U8 = mybir.dt.uint8
AF = mybir.ActivationFunctionType
OP = mybir.AluOpType
AX = mybir.AxisListType

NCORES = 8
LC = 1024
CP = 128          # pair channels
CS = 512          # single channels
BB = 64           # distance bins
H = 8             # heads
DH = 64
W_PAE, W_PDE, EPS = 0.25, 0.1, 1e-6
BF16 = ml_dtypes.bfloat16


def build_program(lc=LC, ablate=()):
    """Per-core Bass program (identical on all cores)."""
    ablate = frozenset(ablate)
    Q = lc // NCORES          # query rows per core
    K = lc                    # keys (full)
    KB = K // 128             # 128-key blocks
    NKO = K // 8              # z tiles (k-octets)
    NDT = K // 16             # dist tiles (16 k each)
    NG = K // 64              # 64-key combine groups
    NH = (K + 511) // 512     # score halves
    SW = min(512, K)

    nc = bacc.Bacc("TRN2", target_bir_lowering=False, debug=False,
                   num_devices=NCORES)

    z_s = nc.dram_tensor("z_s", [Q, K, CP], FP, kind="ExternalInput")
    d_s = nc.dram_tensor("d_s", [Q, K, BB], FP, kind="ExternalInput")
    pc_s = nc.dram_tensor("pc_s", [Q, K], FP, kind="ExternalInput")
    pa_s = nc.dram_tensor("pa_s", [Q, K], FP, kind="ExternalInput")
    pd_s = nc.dram_tensor("pd_s", [Q, K], FP, kind="ExternalInput")
    mk_s = nc.dram_tensor("mk_s", [Q, K], U8, kind="ExternalInput")
    sprox = nc.dram_tensor("sprox", [lc, CS], FP, kind="ExternalInput")
    sown = nc.dram_tensor("sown", [Q, CS], FP, kind="ExternalInput")
    wqe_d = nc.dram_tensor("wqe", [CS + 1, CS], BF, kind="ExternalInput")
    wke_d = nc.dram_tensor("wke", [CS + 1, CS], BF, kind="ExternalInput")
    wve_d = nc.dram_tensor("wve", [CS + 1, CS], BF, kind="ExternalInput")
    w8g_d = nc.dram_tensor("w8g", [CP, H], BF, kind="ExternalInput")
    w2b_d = nc.dram_tensor("w2b", [CP, 2 * H], BF, kind="ExternalInput")
    a2_d = nc.dram_tensor("a2rep", [128, H], FP, kind="ExternalInput")
    bzq_d = nc.dram_tensor("bzq", [H, Q], BF, kind="ExternalInput")
    ones_d = nc.dram_tensor("onesk", [1, K], BF, kind="ExternalInput")
    gbs_d = nc.dram_tensor("gbs", [CS, 2], FP, kind="ExternalInput")

    attn_o = nc.dram_tensor("attn_o", [Q, K, H], FP, kind="ExternalOutput")
    pool_o = nc.dram_tensor("pool_o", [1, CS], FP, kind="ExternalOutput")

    with tile.TileContext(nc) as tc:
        consts = tc.alloc_tile_pool(name="consts", bufs=1)
        id128 = consts.tile([128, 128], BF)
        make_identity(nc, id128)
        w8g = consts.tile([CP, H], BF)
        nc.sync.dma_start(w8g[:], w8g_d.ap())
        w2b = consts.tile([CP, 2 * H], BF)
        nc.sync.dma_start(w2b[:], w2b_d.ap())
        a2rep = consts.tile([128, H], FP)
        nc.sync.dma_start(a2rep[:], a2_d.ap())
        onesk = consts.tile([1, K], BF)
        nc.sync.dma_start(onesk[:], ones_d.ap())
        eps5 = consts.tile([128, 1], FP)
        nc.gpsimd.memset(eps5[:], 1e-5)
        idf = consts.tile([1, 1], FP)
        nc.gpsimd.memset(idf[:], 1.0)
        eps6 = consts.tile([128, 1], FP)
        nc.gpsimd.memset(eps6[:], EPS)
        wq_c, wk_c, wv_c = [], [], []
        for nm, src, lst in (("wq", wqe_d, wq_c), ("wk", wke_d, wk_c),
                             ("wv", wve_d, wv_c)):
            for j in range(4):
                t = consts.tile([128, CS], BF, name=f"{nm}c{j}")
                nc.sync.dma_start(t[:], src.ap()[j * 128:(j + 1) * 128, :])
                lst.append(t)
            t = consts.tile([1, CS], BF, name=f"{nm}b")
            nc.sync.dma_start(t[:], src.ap()[CS:CS + 1, :])
            lst.append(t)

        persist = tc.alloc_tile_pool(name="persist", bufs=1)
        BFULL = persist.tile([Q, K, H], FP)
        OUT = persist.tile([Q, K, H], FP)
        MS = persist.tile([Q, K], FP)      # mean_even + mean_odd per pair
        RS = persist.tile([Q, K], FP)      # 1/sqrt(var+eps) per pair
        PM = persist.tile([Q, K], FP)      # prior + mask additive map
        PROT = persist.tile([Q, 1], FP)
        V8 = persist.tile([128, K // 128, CS], BF)
        QTH = [persist.tile([DH + 1, Q], BF, name=f"qth{h}") for h in range(H)]
        KTH = [persist.tile([DH + 1, K], BF, name=f"kth{h}") for h in range(H)]
        PVT = persist.tile([128, KB * H], BF)
        snT = [persist.tile([128, lc], BF, name=f"snt{j}") for j in range(4)]

        # ============== Phase 0: s-path, projections, P-map ==============
        with tc.tile_pool(name="p0sb", bufs=2) as p0sb, \
             tc.tile_pool(name="p0ps", bufs=1, space="PSUM") as p0ps:
            snt_ps = [p0ps.tile([128, lc], BF, name=f"sntp{j}", bufs=1)
                      for j in range(4)]
            nsc = lc // 128
            sntown = p0sb.tile([128, 4, Q], BF, bufs=1)
            sntown_ps = p0ps.tile([128, Q], BF, bufs=1)
            for i in range(nsc + 1):
                own = i == nsc
                rows = Q if own else 128
                st = p0sb.tile([128, CS], FP, tag="st")
                nc.sync.dma_start(
                    st[:rows, :],
                    sown.ap() if own else sprox.ap()[i * 128:(i + 1) * 128, :])
                sst = p0sb.tile([128, 6], FP, tag="sst")
                nc.vector.bn_stats(sst[:rows, :], st[:rows, :])
                mr = p0sb.tile([128, 4], FP, tag="mr")
                r_ = slice(0, rows)
                nc.vector.tensor_tensor(mr[r_, 0:1], sst[r_, 1:2], sst[r_, 4:5], OP.add)
                nc.vector.tensor_tensor(mr[r_, 1:2], sst[r_, 1:2], sst[r_, 4:5], OP.subtract)
                nc.vector.tensor_tensor(mr[r_, 2:3], sst[r_, 2:3], sst[r_, 5:6], OP.add)
                nc.vector.tensor_scalar(mr[r_, 0:1], mr[r_, 0:1], 0.5, None, OP.mult)
                nc.vector.tensor_tensor(mr[r_, 1:2], mr[r_, 1:2], mr[r_, 1:2], OP.mult)
                nc.vector.tensor_scalar(mr[r_, 1:2], mr[r_, 1:2], 0.25, None, OP.mult)
                nc.vector.tensor_scalar(mr[r_, 2:3], mr[r_, 2:3], 1.0 / CS, None, OP.mult)
                nc.vector.tensor_tensor(mr[r_, 2:3], mr[r_, 2:3], mr[r_, 1:2], OP.add)
                nc.scalar.activation(mr[r_, 3:4], mr[r_, 2:3], AF.Sqrt,
                                     bias=eps5[r_, :])
                rr = p0sb.tile([128, 1], FP, tag="rr")
                nc.vector.reciprocal(rr[r_, :], mr[r_, 3:4])
                snorm = p0sb.tile([128, CS], BF, tag="snorm")
                nc.vector.tensor_scalar(snorm[r_, :], st[r_, :], mr[r_, 0:1],
                                        rr[r_, :], OP.subtract, OP.mult)
                for j in range(4):
                    if own:
                        nc.tensor.matmul(sntown_ps[:, 0:Q],
                                         snorm[0:Q, j * 128:(j + 1) * 128],
                                         id128[0:Q, 0:Q], is_transpose=True,
                                         start=True, stop=True)
                        nc.scalar.copy(sntown[:, j, :], sntown_ps[:, 0:Q])
                    else:
                        nc.tensor.matmul(snt_ps[j][:, i * 128:(i + 1) * 128],
                                         snorm[:, j * 128:(j + 1) * 128],
                                         id128[:], is_transpose=True,
                                         start=(i == 0), stop=(i == nsc - 1))
            for j in range(4):
                gb = p0sb.tile([128, 2], FP, tag="gb", bufs=4)
                nc.sync.dma_start(gb[:], gbs_d.ap()[j * 128:(j + 1) * 128, :])
                nc.vector.tensor_scalar(snT[j][:], snt_ps[j][:], gb[:, 0:1],
                                        gb[:, 1:2], OP.mult, OP.add)
                nc.vector.tensor_scalar(sntown[:, j, :], sntown[:, j, :],
                                        gb[:, 0:1], gb[:, 1:2], OP.mult, OP.add)

            for h in range(H):
                c0 = h * DH
                for half in range(NH):
                    sl = slice(half * 512, half * 512 + SW)
                    kp = p0ps.tile([DH, SW], FP, tag="kp")
                    for j in range(4):
                        nc.tensor.matmul(kp[:], wk_c[j][:, c0:c0 + DH],
                                         snT[j][:, sl], start=(j == 0), stop=False)
                    nc.tensor.matmul(kp[:], wk_c[4][0:1, c0:c0 + DH],
                                     onesk[:, sl], start=False, stop=True)
                    nc.scalar.copy(KTH[h][0:DH, sl], kp[:])
                nc.sync.dma_start(KTH[h][DH:DH + 1, :], ones_d.ap())

                qp = p0ps.tile([DH, Q], FP, tag="qp")
                for j in range(4):
                    nc.tensor.matmul(qp[:], wq_c[j][:, c0:c0 + DH],
                                     sntown[:, j, :], start=(j == 0), stop=False)
                nc.tensor.matmul(qp[:], wq_c[4][0:1, c0:c0 + DH],
                                 onesk[:, 0:Q], start=False, stop=True)
                nc.vector.tensor_copy(QTH[h][0:DH, :], qp[:])
                nc.sync.dma_start(QTH[h][DH:DH + 1, :], bzq_d.ap()[h:h + 1, :])

            for i in range(K // 128):
                vp = p0ps.tile([128, CS], FP, tag="vp")
                for j in range(4):
                    nc.tensor.matmul(vp[:], snT[j][:, i * 128:(i + 1) * 128],
                                     wv_c[j][:], start=(j == 0), stop=False)
                nc.tensor.matmul(vp[:], onesk[:, i * 128:(i + 1) * 128],
                                 wv_c[4][:], start=False, stop=True)
                nc.scalar.copy(V8[:, i, :], vp[:])

            # prior + mask map and prot
            lg = [p0sb.tile([Q, K], FP, name=f"lg{t}", tag="lg", bufs=3)
                  for t in range(3)]
            for t, src in enumerate([pc_s, pa_s, pd_s]):
                raw = p0sb.tile([Q, K], FP, tag="praw")
                nc.sync.dma_start(raw[:], src.ap())
                nc.scalar.activation(lg[t][:], raw[:], AF.Ln, bias=eps6[0:Q, :])
            nc.vector.scalar_tensor_tensor(PM[:], lg[1][:], -W_PAE, lg[0][:],
                                           OP.mult, OP.add)
            nc.vector.scalar_tensor_tensor(PM[:], lg[2][:], -W_PDE, PM[:],
                                           OP.mult, OP.add)
            mraw = p0sb.tile([Q, K], U8, tag="mraw")
            nc.sync.dma_start(mraw[:], mk_s.ap())
            mterm = p0sb.tile([Q, K], FP, tag="mterm")
            nc.vector.tensor_scalar(mterm[:], mraw[:], 1e9, -1e9, OP.mult, OP.add)
            nc.vector.tensor_tensor(PM[:], PM[:], mterm[:], OP.add)
            mmax = p0sb.tile([Q, 1], FP, tag="mmax")
            nc.vector.tensor_reduce(mmax[:], mterm[:], AX.X, OP.max)
            nc.vector.tensor_scalar(PROT[:], mmax[:], 1e-9, 1.0, OP.mult, OP.add)

        # ============== Phase 1: z & dist streams ========================
        with tc.tile_pool(name="p1sb", bufs=3) as p1sb, \
             tc.tile_pool(name="stat", bufs=2) as statp, \
             tc.tile_pool(name="p1ps", bufs=2, space="PSUM") as p1ps:
            stk = zwp = dwp = None
            zw_hist, dw_hist = {}, {}
            for ko in range(NKO):
                kb = ko // 16
                zt = p1sb.tile([Q, 8, CP], BF, tag="zt")
                nc.gpsimd.dma_start(zt[:], z_s.ap()[:, ko * 8:(ko + 1) * 8, :])
                if ko % 16 == 0:
                    stk = statp.tile([Q, 128, 6], FP, tag="stk")
                so = (ko % 16) * 8
                if "nostats" not in ablate:
                    # HW BNStats emits exactly 6 values/partition per call
                    for j in range(8):
                        nc.vector.bn_stats(stk[:, so + j, :], zt[:, j, :])
                elif ko % 16 == 0:
                    nc.vector.memset(stk[:], 1.0)
                if "noztr" not in ablate:
                    ztp = p1ps.tile([128, 8, Q], BF, tag="ztp")
                    for j in range(8):
                        nc.tensor.matmul(ztp[:, j, :], zt[:, j, :],
                                         id128[0:Q, 0:Q], is_transpose=True,
                                         start=(j == 0), stop=(j == 7))
                    zTs = p1sb.tile([128, 8, Q], BF, tag="zTs")
                    nc.scalar.copy(zTs[:], ztp[:])
                elif ko == 0:
                    zTs = p1sb.tile([128, 8, Q], BF, tag="zTs")
                    nc.vector.memset(zTs[:], 0.5)
                if ko % 8 == 0:
                    zwp = p1ps.tile([Q, 512], FP, tag="zwp")
                for j in range(8):
                    k = ko * 8 + j
                    nc.tensor.matmul(zwp[:, (k % 64) * 8:(k % 64) * 8 + 8],
                                     zTs[:, j, :], w8g[:],
                                     start=(k % 64 == 0), stop=(k % 64 == 63))
                if ko % 8 == 7:
                    zw_hist[ko // 8] = zwp

                # dist: one 16-k tile per two z octets
                if ko % 2 == 1 and "nodist" not in ablate:
                    dti = ko // 2
                    dt = p1sb.tile([Q, 16, BB], BF, tag="dt")
                    nc.gpsimd.dma_start(dt[:], d_s.ap()[:, dti * 16:(dti + 1) * 16, :])
                    dtv = dt.rearrange("q (p t) b -> q p (t b)", t=2)
                    dtp = p1ps.tile([128, 8, Q], BF, tag="dtp")
                    for p in range(8):
                        nc.tensor.matmul(dtp[:, p, :], dtv[:, p, :],
                                         id128[0:Q, 0:Q], is_transpose=True,
                                         start=(p == 0), stop=(p == 7))
                    dTs = p1sb.tile([128, 8, Q], BF, tag="dTs")
                    nc.scalar.copy(dTs[:], dtp[:])
                    if dti % 4 == 0:
                        dwp = p1ps.tile([Q, 512], FP, tag="dwp")
                    for p in range(8):
                        kp_i = dti * 8 + p
                        nc.tensor.matmul(dwp[:, (kp_i % 32) * 16:(kp_i % 32) * 16 + 16],
                                         dTs[:, p, :], w2b[:],
                                         start=(kp_i % 32 == 0), stop=(kp_i % 32 == 31))

                    if dti % 4 == 3:
                        dw_hist[dti // 4] = dwp

                if ko % 16 == 15:
                    sl = slice(kb * 128, (kb + 1) * 128)
                    tA = p1sb.tile([Q, 128], FP, tag="tA")
                    tB = p1sb.tile([Q, 128], FP, tag="tB")
                    nc.vector.tensor_tensor(MS[:, sl], stk[:, :, 1], stk[:, :, 4], OP.add)
                    nc.vector.tensor_tensor(tA[:], stk[:, :, 1], stk[:, :, 4], OP.subtract)
                    nc.vector.tensor_tensor(tB[:], stk[:, :, 2], stk[:, :, 5], OP.add)
                    nc.vector.tensor_tensor(tA[:], tA[:], tA[:], OP.mult)
                    nc.vector.tensor_scalar(tA[:], tA[:], 0.25, None, OP.mult)
                    nc.vector.tensor_scalar(tB[:], tB[:], 1.0 / CP, None, OP.mult)
                    nc.vector.tensor_tensor(tB[:], tB[:], tA[:], OP.add)
                    nc.scalar.activation(tB[:], tB[:], AF.Sqrt, bias=eps5[0:Q, :])
                    nc.vector.reciprocal(RS[:, sl], tB[:])
                    if "nocomb" in ablate:
                        continue
                    for g in (2 * kb, 2 * kb + 1):
                        ks = slice(g * 64, (g + 1) * 64)
                        msb = MS[:, ks][:, :, None].broadcast_to([Q, 64, H])
                        rsb = RS[:, ks][:, :, None].broadcast_to([Q, 64, H])
                        pmb = PM[:, ks][:, :, None].broadcast_to([Q, 64, H])
                        a2b = a2rep[0:Q, None, :].broadcast_to([Q, 64, H])
                        tC = p1sb.tile([Q, 64, H], FP, tag="tC")
                        zwv = zw_hist.pop(g).rearrange("q (k h) -> q k h", h=H)
                        if "nodist" not in ablate:
                            dwv = dw_hist.pop(g).rearrange("q (k h) -> q k h", h=H)
                        nc.vector.tensor_tensor(tC[:], msb, a2b, OP.mult)
                        nc.vector.tensor_tensor(tC[:], zwv, tC[:], OP.subtract)
                        nc.vector.tensor_tensor(tC[:], tC[:], rsb, OP.mult)
                        if "nodist" not in ablate:
                            nc.vector.tensor_tensor(tC[:], tC[:], dwv, OP.add)
                        nc.vector.tensor_tensor(BFULL[:, ks, :], tC[:], pmb, OP.add)

        # ============== Phase 2: scores / softmax / outputs ==============
        with tc.tile_pool(name="p2sb", bufs=2) as p2sb, \
             tc.tile_pool(name="p2ps", bufs=2, space="PSUM") as p2ps, \
             tc.tile_pool(name="p2ps1", bufs=1, space="PSUM") as p2ps1:
            pvtp = p2ps1.tile([128, KB * H], FP)
            plp = p2ps1.tile([1, CS], FP)
            for h in range(H):
                scp = p2ps.tile([Q, K], FP, tag="scp")
                for half in range(NH):
                    sl = slice(half * 512, half * 512 + SW)
                    nc.tensor.matmul(scp[:, sl], QTH[h][:], KTH[h][:, sl],
                                     start=True, stop=True)
                nc.vector.tensor_tensor(scp[:], scp[:], BFULL[:, :, h], OP.add)
                eh = p2sb.tile([Q, K], BF, tag="eh")
                zs = p2sb.tile([Q, 1], FP, tag="zs")
                nc.scalar.activation(eh[:], scp[:], AF.Exp, accum_out=zs[:])
                rz = p2sb.tile([Q, 1], FP, tag="rz")
                nc.vector.reciprocal(rz[:], zs[:])
                nc.scalar.mul(OUT[:, :, h], eh[:], rz[:])
                pz = p2sb.tile([Q, 1], BF, tag="pz")
                nc.vector.tensor_tensor(pz[:], PROT[:], rz[:], OP.mult)
                pvp = p2ps.tile([1, K], FP, tag="pvp", bufs=1)
                for half in range(NH):
                    sl = slice(half * 512, half * 512 + SW)
                    nc.tensor.matmul(pvp[0:1, sl], pz[:], eh[:, sl],
                                     start=True, stop=True)
                pve = p2sb.tile([1, K], FP, tag="pve")
                nc.scalar.copy(pve[:], pvp[:])
                for kc in range(KB):
                    i = h * KB + kc
                    nc.tensor.matmul(pvtp[:, i:i + 1],
                                     pve[0:1, kc * 128:(kc + 1) * 128],
                                     idf[0:1, 0:1], is_transpose=True,
                                     start=(i == 0), stop=(i == H * KB - 1))
            nc.vector.tensor_copy(PVT[:], pvtp[:])
            for h in range(H):
                for kc in range(KB):
                    nc.tensor.matmul(plp[0:1, h * DH:(h + 1) * DH],
                                     PVT[:, h * KB + kc:h * KB + kc + 1],
                                     V8[:, kc, h * DH:(h + 1) * DH],
                                     start=(h == 0 and kc == 0),
                                     stop=(h == H - 1 and kc == KB - 1))
            plo = p2sb.tile([1, CS], FP)
            nc.scalar.copy(plo[:], plp[:])
            nc.sync.dma_start(pool_o.ap(), plo[:])
            nc.sync.dma_start(attn_o.ap(), OUT[:])

        persist.release()
        consts.release()

    nc.compile()
    return nc


def host_prep(inputs, lc=LC):
    """Build per-core input maps from the full inputs."""
    f32 = np.float32
    z = np.ascontiguousarray(np.asarray(inputs["z"], f32))
    dist = np.ascontiguousarray(np.asarray(inputs["dist_bins"], f32))
    mask = np.asarray(inputs["pd_mask"])
    pc = np.asarray(inputs["prior_contact"], f32)
    pa = np.asarray(inputs["prior_pae"], f32)
    pd = np.asarray(inputs["prior_pde"], f32)
    sp = np.asarray(inputs["s_proxy"], f32)

    g = np.asarray(inputs["ln_z_g"], f32)
    b = np.asarray(inputs["ln_z_b"], f32)
    w_zb = np.asarray(inputs["w_zb"], f32)
    w8g = np.ascontiguousarray((g[:, None] * w_zb).astype(BF16))
    A2 = (g[:, None] * w_zb).sum(axis=0) * 0.5            # A_h / 2
    a2rep = np.ascontiguousarray(np.broadcast_to(A2.astype(f32), (128, H)))
    Bz = (b[:, None] * w_zb).sum(axis=0).astype(f32)      # [H]

    wd = np.asarray(inputs["w_dist"], f32)
    w2b = np.zeros((CP, 2 * H), f32)
    w2b[:BB, :H] = wd
    w2b[BB:, H:] = wd
    w2b = w2b.astype(BF16)

    sc = f32(1.0) / np.sqrt(f32(DH))

    def ext(w, bias, scale=f32(1.0)):
        e = np.zeros((CS + 1, CS), f32)
        e[:CS] = np.asarray(w, f32) * scale
        e[CS] = np.asarray(bias, f32) * scale
        return np.ascontiguousarray(e.astype(BF16))

    wqe = ext(inputs["w_q"], inputs["b_q"], sc)
    wke = ext(inputs["w_k"], inputs["b_k"])
    wve = ext(inputs["w_v"], inputs["b_v"])

    Qr = lc // NCORES
    bzq = np.ascontiguousarray(np.broadcast_to(Bz[:, None].astype(BF16), (H, Qr)))
    onesk = np.ones((1, lc), BF16)
    gbs = np.ascontiguousarray(np.stack(
        [np.asarray(inputs["ln_s_g"], f32), np.asarray(inputs["ln_s_b"], f32)],
        axis=1))

    maps = []
    for c in range(NCORES):
        sl = slice(c * Qr, (c + 1) * Qr)
        maps.append({
            "z_s": z[sl], "d_s": dist[sl],
            "pc_s": pc[sl], "pa_s": pa[sl], "pd_s": pd[sl],
            "mk_s": mask[sl].astype(np.uint8),
            "sprox": sp, "sown": np.ascontiguousarray(sp[sl]),
            "wqe": wqe, "wke": wke, "wve": wve,
            "w8g": w8g, "w2b": w2b, "a2rep": a2rep, "bzq": bzq,
            "onesk": onesk, "gbs": gbs,
        })
    return maps


def host_finish(results, inputs):
    """Gather per-core results; tiny output MLP on the host (f32)."""
    f32 = np.float32
    attn = np.concatenate([np.asarray(r["attn_o"], f32) for r in results], axis=0)
    PP = np.sum([np.asarray(r["pool_o"][0], f32) for r in results],
                axis=0, dtype=f32)

    mask = np.asarray(inputs["pd_mask"])
    sp_ = f32(mask.any(axis=1).sum())
    cnt = np.maximum(sp_, f32(1.0))
    w_o = np.asarray(inputs["w_o"], f32)
    b_o = np.asarray(inputs["b_o"], f32)
    pooled = (PP @ w_o + sp_ * b_o) / cnt
    m = pooled.mean(dtype=f32)
    v = f32(np.mean(np.square(pooled - m), dtype=f32))
    hn = (pooled - m) / np.sqrt(v + f32(1e-5))
    hn = hn * np.asarray(inputs["mlp_ln_g"], f32) + np.asarray(inputs["mlp_ln_b"], f32)
    a = np.maximum(hn @ np.asarray(inputs["mlp_w1"], f32)
                   + np.asarray(inputs["mlp_b1"], f32), f32(0.0))
    aff = (a @ np.asarray(inputs["mlp_w2"], f32)
           + np.asarray(inputs["mlp_b2"], f32)).reshape(())
    return np.asarray(aff, f32), attn


_NC_CACHE = {}


def _get_nc(lc=LC):
    if lc not in _NC_CACHE:
        _NC_CACHE[lc] = build_program(lc)
    return _NC_CACHE[lc]


def kernel(**inputs):
    nc = _get_nc()
    in_maps = host_prep(inputs)
    res = run_bass_kernel_spmd(nc, in_maps, core_ids=list(range(NCORES)))
    return host_finish(res.results, inputs)


# revision 32
# speedup vs baseline: 1.0811x; 1.0811x over previous
"""Trainium2 Bass kernel for the BoltzAffinityHead replica.

Sequence-parallel over query rows: each of the 8 cores gets LC/8 = 128
query rows of z / dist_bins / priors / pd_mask; s_proxy and params are
replicated. Each core emits its attn_w shard plus a partial masked-pooled
sum; the tiny output MLP runs on the host after summing per-core partials
(the all-reduce step of the sharding hint).

Per-core pipeline:
  * q/k/v projections from LN(s_proxy); biases folded in via an extra
    contraction row, 1/sqrt(dh) folded into w_q, and the constant part of
    the pair-bias LayerNorm folded in via a 65th contraction row of Q/K.
  * z streamed as bf16 [q,(k,c)] tiles; bn_stats gives LN mean/var; PE
    transposes make z^T slices which are then used as the *stationary*
    matmul operand against g*w_zb so zW lands directly in [q,(k,h)]
    layout; dist_bins handled the same way with block-diagonal weights
    (two k per matmul).
  * bias map B[q,k,h] = (zW - mean*A')*rsqrt(var+eps) + dW + prior+mask,
    built with full-rate DVE map ops; scores = QK + B in PSUM; exp on ACT
    with fused denominator accumulation (max-free softmax - logits are
    bounded by construction); per-head interleave into the output tile
    with the 1/Z scale fused.
  * pooled partial: prot-weighted column reduction of exp(scores) on the
    PE, transposed, then a tiny matvec with V.
"""

import numpy as np
import ml_dtypes

import concourse.bass as bass
import concourse.tile as tile
from concourse import bacc, mybir
from concourse.bass_utils import run_bass_kernel_spmd
from concourse.masks import make_identity

FP = mybir.dt.float32
BF = mybir.dt.bfloat16
U8 = mybir.dt.uint8
AF = mybir.ActivationFunctionType
OP = mybir.AluOpType
AX = mybir.AxisListType

NCORES = 8
LC = 1024
CP = 128          # pair channels
CS = 512          # single channels
BB = 64           # distance bins
H = 8             # heads
DH = 64
W_PAE, W_PDE, EPS = 0.25, 0.1, 1e-6
BF16 = ml_dtypes.bfloat16


KNOBS = {"p1bufs": 6, "psbufs": 2, "evac_dve": 0, "scpbufs": 2}


def build_program(lc=LC, ablate=(), **kn):
    """Per-core Bass program (identical on all cores)."""
    ablate = frozenset(ablate)
    kb_ = dict(KNOBS)
    kb_.update(kn)
    Q = lc // NCORES          # query rows per core
    K = lc                    # keys (full)
    KB = K // 128             # 128-key blocks
    NKO = K // 8              # z tiles (k-octets)
    NDT = K // 16             # dist tiles (16 k each)
    NG = K // 64              # 64-key combine groups
    NH = (K + 511) // 512     # score halves
    SW = min(512, K)

    nc = bacc.Bacc("TRN2", target_bir_lowering=False, debug=False,
                   num_devices=NCORES)

    z_s = nc.dram_tensor("z_s", [Q, K, CP], FP, kind="ExternalInput")
    d_s = nc.dram_tensor("d_s", [Q, K, BB], FP, kind="ExternalInput")
    pc_s = nc.dram_tensor("pc_s", [Q, K], FP, kind="ExternalInput")
    pa_s = nc.dram_tensor("pa_s", [Q, K], FP, kind="ExternalInput")
    pd_s = nc.dram_tensor("pd_s", [Q, K], FP, kind="ExternalInput")
    mk_s = nc.dram_tensor("mk_s", [Q, K], U8, kind="ExternalInput")
    sprox = nc.dram_tensor("sprox", [lc, CS], FP, kind="ExternalInput")
    sown = nc.dram_tensor("sown", [Q, CS], FP, kind="ExternalInput")
    wqe_d = nc.dram_tensor("wqe", [CS + 1, CS], BF, kind="ExternalInput")
    wke_d = nc.dram_tensor("wke", [CS + 1, CS], BF, kind="ExternalInput")
    wve_d = nc.dram_tensor("wve", [CS + 1, CS], BF, kind="ExternalInput")
    w8g_d = nc.dram_tensor("w8g", [CP, H], BF, kind="ExternalInput")
    w2b_d = nc.dram_tensor("w2b", [CP, 2 * H], BF, kind="ExternalInput")
    a2_d = nc.dram_tensor("a2rep", [128, H], FP, kind="ExternalInput")
    bzq_d = nc.dram_tensor("bzq", [H, Q], BF, kind="ExternalInput")
    ones_d = nc.dram_tensor("onesk", [1, K], BF, kind="ExternalInput")
    gbs_d = nc.dram_tensor("gbs", [CS, 2], FP, kind="ExternalInput")

    attn_o = nc.dram_tensor("attn_o", [Q, K, H], FP, kind="ExternalOutput")
    pool_o = nc.dram_tensor("pool_o", [1, CS], FP, kind="ExternalOutput")

    with tile.TileContext(nc) as tc:
        consts = tc.alloc_tile_pool(name="consts", bufs=1)
        id128 = consts.tile([128, 128], BF)
        make_identity(nc, id128)
        w8g = consts.tile([CP, H], BF)
        nc.sync.dma_start(w8g[:], w8g_d.ap())
        w2b = consts.tile([CP, 2 * H], BF)
        nc.sync.dma_start(w2b[:], w2b_d.ap())
        a2rep = consts.tile([128, H], FP)
        nc.sync.dma_start(a2rep[:], a2_d.ap())
        onesk = consts.tile([1, K], BF)
        nc.sync.dma_start(onesk[:], ones_d.ap())
        eps5 = consts.tile([128, 1], FP)
        nc.gpsimd.memset(eps5[:], 1e-5)
        idf = consts.tile([1, 1], FP)
        nc.gpsimd.memset(idf[:], 1.0)
        eps6 = consts.tile([128, 1], FP)
        nc.gpsimd.memset(eps6[:], EPS)
        wq_c, wk_c, wv_c = [], [], []
        for nm, src, lst in (("wq", wqe_d, wq_c), ("wk", wke_d, wk_c),
                             ("wv", wve_d, wv_c)):
            for j in range(4):
                t = consts.tile([128, CS], BF, name=f"{nm}c{j}")
                nc.sync.dma_start(t[:], src.ap()[j * 128:(j + 1) * 128, :])
                lst.append(t)
            t = consts.tile([1, CS], BF, name=f"{nm}b")
            nc.sync.dma_start(t[:], src.ap()[CS:CS + 1, :])
            lst.append(t)

        persist = tc.alloc_tile_pool(name="persist", bufs=1)
        BFULL = persist.tile([Q, K, H], FP)
        OUT = persist.tile([Q, K, H], FP)
        MS = persist.tile([Q, K], FP)      # mean_even + mean_odd per pair
        RS = persist.tile([Q, K], FP)      # 1/sqrt(var+eps) per pair
        PM = persist.tile([Q, K], FP)      # prior + mask additive map
        PROT = persist.tile([Q, 1], FP)
        V8 = persist.tile([128, K // 128, CS], BF)
        QTH = [persist.tile([DH + 1, Q], BF, name=f"qth{h}") for h in range(H)]
        KTH = [persist.tile([DH + 1, K], BF, name=f"kth{h}") for h in range(H)]
        PVT = persist.tile([128, KB * H], BF)
        snT = [persist.tile([128, lc], BF, name=f"snt{j}") for j in range(4)]

        # ============== Phase 0: s-path, projections, P-map ==============
        with tc.tile_pool(name="p0sb", bufs=2) as p0sb, \
             tc.tile_pool(name="p0ps", bufs=1, space="PSUM") as p0ps:
            snt_ps = [p0ps.tile([128, lc], BF, name=f"sntp{j}", bufs=1)
                      for j in range(4)]
            nsc = lc // 128
            sntown = p0sb.tile([128, 4, Q], BF, bufs=1)
            sntown_ps = p0ps.tile([128, Q], BF, bufs=1)
            for i in range(nsc + 1):
                own = i == nsc
                rows = Q if own else 128
                st = p0sb.tile([128, CS], FP, tag="st")
                nc.sync.dma_start(
                    st[:rows, :],
                    sown.ap() if own else sprox.ap()[i * 128:(i + 1) * 128, :])
                sst = p0sb.tile([128, 6], FP, tag="sst")
                nc.vector.bn_stats(sst[:rows, :], st[:rows, :])
                mr = p0sb.tile([128, 4], FP, tag="mr")
                r_ = slice(0, rows)
                nc.vector.tensor_tensor(mr[r_, 0:1], sst[r_, 1:2], sst[r_, 4:5], OP.add)
                nc.vector.tensor_tensor(mr[r_, 1:2], sst[r_, 1:2], sst[r_, 4:5], OP.subtract)
                nc.vector.tensor_tensor(mr[r_, 2:3], sst[r_, 2:3], sst[r_, 5:6], OP.add)
                nc.vector.tensor_scalar(mr[r_, 0:1], mr[r_, 0:1], 0.5, None, OP.mult)
                nc.vector.tensor_tensor(mr[r_, 1:2], mr[r_, 1:2], mr[r_, 1:2], OP.mult)
                nc.vector.tensor_scalar(mr[r_, 1:2], mr[r_, 1:2], 0.25, None, OP.mult)
                nc.vector.tensor_scalar(mr[r_, 2:3], mr[r_, 2:3], 1.0 / CS, None, OP.mult)
                nc.vector.tensor_tensor(mr[r_, 2:3], mr[r_, 2:3], mr[r_, 1:2], OP.add)
                nc.scalar.activation(mr[r_, 3:4], mr[r_, 2:3], AF.Sqrt,
                                     bias=eps5[r_, :])
                rr = p0sb.tile([128, 1], FP, tag="rr")
                nc.vector.reciprocal(rr[r_, :], mr[r_, 3:4])
                snorm = p0sb.tile([128, CS], BF, tag="snorm")
                nc.vector.tensor_scalar(snorm[r_, :], st[r_, :], mr[r_, 0:1],
                                        rr[r_, :], OP.subtract, OP.mult)
                for j in range(4):
                    if own:
                        nc.tensor.matmul(sntown_ps[:, 0:Q],
                                         snorm[0:Q, j * 128:(j + 1) * 128],
                                         id128[0:Q, 0:Q], is_transpose=True,
                                         start=True, stop=True)
                        nc.scalar.copy(sntown[:, j, :], sntown_ps[:, 0:Q])
                    else:
                        nc.tensor.matmul(snt_ps[j][:, i * 128:(i + 1) * 128],
                                         snorm[:, j * 128:(j + 1) * 128],
                                         id128[:], is_transpose=True,
                                         start=(i == 0), stop=(i == nsc - 1))
            for j in range(4):
                gb = p0sb.tile([128, 2], FP, tag="gb", bufs=4)
                nc.sync.dma_start(gb[:], gbs_d.ap()[j * 128:(j + 1) * 128, :])
                nc.vector.tensor_scalar(snT[j][:], snt_ps[j][:], gb[:, 0:1],
                                        gb[:, 1:2], OP.mult, OP.add)
                nc.vector.tensor_scalar(sntown[:, j, :], sntown[:, j, :],
                                        gb[:, 0:1], gb[:, 1:2], OP.mult, OP.add)

            for h in range(H):
                c0 = h * DH
                for half in range(NH):
                    sl = slice(half * 512, half * 512 + SW)
                    kp = p0ps.tile([DH, SW], FP, tag="kp")
                    for j in range(4):
                        nc.tensor.matmul(kp[:], wk_c[j][:, c0:c0 + DH],
                                         snT[j][:, sl], start=(j == 0), stop=False)
                    nc.tensor.matmul(kp[:], wk_c[4][0:1, c0:c0 + DH],
                                     onesk[:, sl], start=False, stop=True)
                    nc.scalar.copy(KTH[h][0:DH, sl], kp[:])
                nc.sync.dma_start(KTH[h][DH:DH + 1, :], ones_d.ap())

                qp = p0ps.tile([DH, Q], FP, tag="qp")
                for j in range(4):
                    nc.tensor.matmul(qp[:], wq_c[j][:, c0:c0 + DH],
                                     sntown[:, j, :], start=(j == 0), stop=False)
                nc.tensor.matmul(qp[:], wq_c[4][0:1, c0:c0 + DH],
                                 onesk[:, 0:Q], start=False, stop=True)
                nc.vector.tensor_copy(QTH[h][0:DH, :], qp[:])
                nc.sync.dma_start(QTH[h][DH:DH + 1, :], bzq_d.ap()[h:h + 1, :])

            for i in range(K // 128):
                vp = p0ps.tile([128, CS], FP, tag="vp")
                for j in range(4):
                    nc.tensor.matmul(vp[:], snT[j][:, i * 128:(i + 1) * 128],
                                     wv_c[j][:], start=(j == 0), stop=False)
                nc.tensor.matmul(vp[:], onesk[:, i * 128:(i + 1) * 128],
                                 wv_c[4][:], start=False, stop=True)
                nc.scalar.copy(V8[:, i, :], vp[:])

            # prior + mask map and prot
            lg = [p0sb.tile([Q, K], FP, name=f"lg{t}", tag="lg", bufs=3)
                  for t in range(3)]
            for t, src in enumerate([pc_s, pa_s, pd_s]):
                raw = p0sb.tile([Q, K], FP, tag="praw")
                nc.sync.dma_start(raw[:], src.ap())
                nc.scalar.activation(lg[t][:], raw[:], AF.Ln, bias=eps6[0:Q, :])
            nc.vector.scalar_tensor_tensor(PM[:], lg[1][:], -W_PAE, lg[0][:],
                                           OP.mult, OP.add)
            nc.vector.scalar_tensor_tensor(PM[:], lg[2][:], -W_PDE, PM[:],
                                           OP.mult, OP.add)
            mraw = p0sb.tile([Q, K], U8, tag="mraw")
            nc.sync.dma_start(mraw[:], mk_s.ap())
            mterm = p0sb.tile([Q, K], FP, tag="mterm")
            nc.vector.tensor_scalar(mterm[:], mraw[:], 1e9, -1e9, OP.mult, OP.add)
            nc.vector.tensor_tensor(PM[:], PM[:], mterm[:], OP.add)
            mmax = p0sb.tile([Q, 1], FP, tag="mmax")
            nc.vector.tensor_reduce(mmax[:], mterm[:], AX.X, OP.max)
            nc.vector.tensor_scalar(PROT[:], mmax[:], 1e-9, 1.0, OP.mult, OP.add)

        # ============== Phase 1: z & dist streams ========================
        with tc.tile_pool(name="p1sb", bufs=kb_["p1bufs"]) as p1sb, \
             tc.tile_pool(name="stat", bufs=2) as statp, \
             tc.tile_pool(name="p1ps", bufs=kb_["psbufs"], space="PSUM") as p1ps:
            stk = zwp = dwp = None
            zw_hist, dw_hist = {}, {}
            for ko in range(NKO):
                kb = ko // 16
                zt = p1sb.tile([Q, 8, CP], BF, tag="zt")
                nc.gpsimd.dma_start(zt[:], z_s.ap()[:, ko * 8:(ko + 1) * 8, :])
                if ko % 16 == 0:
                    stk = statp.tile([Q, 128, 6], FP, tag="stk")
                so = (ko % 16) * 8
                if "nostats" not in ablate:
                    # HW BNStats emits exactly 6 values/partition per call
                    for j in range(8):
                        nc.vector.bn_stats(stk[:, so + j, :], zt[:, j, :])
                elif ko % 16 == 0:
                    nc.vector.memset(stk[:], 1.0)
                if "noztr" not in ablate:
                    ztp = p1ps.tile([128, 8, Q], BF, tag="ztp")
                    for j in range(8):
                        nc.tensor.matmul(ztp[:, j, :], zt[:, j, :],
                                         id128[0:Q, 0:Q], is_transpose=True,
                                         start=(j == 0), stop=(j == 7))
                    zTs = p1sb.tile([128, 8, Q], BF, tag="zTs")
                    if ko % 4 < kb_["evac_dve"]:
                        nc.vector.tensor_copy(zTs[:], ztp[:])
                    else:
                        nc.scalar.copy(zTs[:], ztp[:])
                elif ko == 0:
                    zTs = p1sb.tile([128, 8, Q], BF, tag="zTs")
                    nc.vector.memset(zTs[:], 0.5)
                if ko % 8 == 0:
                    zwp = p1ps.tile([Q, 512], FP, tag="zwp")
                for j in range(8):
                    k = ko * 8 + j
                    nc.tensor.matmul(zwp[:, (k % 64) * 8:(k % 64) * 8 + 8],
                                     zTs[:, j, :], w8g[:],
                                     start=(k % 64 == 0), stop=(k % 64 == 63))
                if ko % 8 == 7:
                    zw_hist[ko // 8] = zwp

                # dist: one 16-k tile per two z octets
                if ko % 2 == 1 and "nodist" not in ablate:
                    dti = ko // 2
                    dt = p1sb.tile([Q, 16, BB], BF, tag="dt")
                    nc.gpsimd.dma_start(dt[:], d_s.ap()[:, dti * 16:(dti + 1) * 16, :])
                    dtv = dt.rearrange("q (p t) b -> q p (t b)", t=2)
                    dtp = p1ps.tile([128, 8, Q], BF, tag="dtp")
                    for p in range(8):
                        nc.tensor.matmul(dtp[:, p, :], dtv[:, p, :],
                                         id128[0:Q, 0:Q], is_transpose=True,
                                         start=(p == 0), stop=(p == 7))
                    dTs = p1sb.tile([128, 8, Q], BF, tag="dTs")
                    nc.scalar.copy(dTs[:], dtp[:])
                    if dti % 4 == 0:
                        dwp = p1ps.tile([Q, 512], FP, tag="dwp")
                    for p in range(8):
                        kp_i = dti * 8 + p
                        nc.tensor.matmul(dwp[:, (kp_i % 32) * 16:(kp_i % 32) * 16 + 16],
                                         dTs[:, p, :], w2b[:],
                                         start=(kp_i % 32 == 0), stop=(kp_i % 32 == 31))

                    if dti % 4 == 3:
                        dw_hist[dti // 4] = dwp

                if ko % 16 == 15:
                    sl = slice(kb * 128, (kb + 1) * 128)
                    tA = p1sb.tile([Q, 128], FP, tag="tA")
                    tB = p1sb.tile([Q, 128], FP, tag="tB")
                    nc.vector.tensor_tensor(MS[:, sl], stk[:, :, 1], stk[:, :, 4], OP.add)
                    nc.vector.tensor_tensor(tA[:], stk[:, :, 1], stk[:, :, 4], OP.subtract)
                    nc.vector.tensor_tensor(tB[:], stk[:, :, 2], stk[:, :, 5], OP.add)
                    nc.vector.tensor_tensor(tA[:], tA[:], tA[:], OP.mult)
                    nc.vector.tensor_scalar(tA[:], tA[:], 0.25, None, OP.mult)
                    nc.vector.tensor_scalar(tB[:], tB[:], 1.0 / CP, None, OP.mult)
                    nc.vector.tensor_tensor(tB[:], tB[:], tA[:], OP.add)
                    nc.scalar.activation(tB[:], tB[:], AF.Sqrt, bias=eps5[0:Q, :])
                    nc.vector.reciprocal(RS[:, sl], tB[:])
                    if "nocomb" in ablate:
                        continue
                    for g in (2 * kb, 2 * kb + 1):
                        ks = slice(g * 64, (g + 1) * 64)
                        msb = MS[:, ks][:, :, None].broadcast_to([Q, 64, H])
                        rsb = RS[:, ks][:, :, None].broadcast_to([Q, 64, H])
                        pmb = PM[:, ks][:, :, None].broadcast_to([Q, 64, H])
                        a2b = a2rep[0:Q, None, :].broadcast_to([Q, 64, H])
                        tC = p1sb.tile([Q, 64, H], FP, tag="tC")
                        zwv = zw_hist.pop(g).rearrange("q (k h) -> q k h", h=H)
                        if "nodist" not in ablate:
                            dwv = dw_hist.pop(g).rearrange("q (k h) -> q k h", h=H)
                        nc.vector.tensor_tensor(tC[:], msb, a2b, OP.mult)
                        nc.vector.tensor_tensor(tC[:], zwv, tC[:], OP.subtract)
                        nc.vector.tensor_tensor(tC[:], tC[:], rsb, OP.mult)
                        if "nodist" not in ablate:
                            nc.vector.tensor_tensor(tC[:], tC[:], dwv, OP.add)
                        nc.vector.tensor_tensor(BFULL[:, ks, :], tC[:], pmb, OP.add)

        # ============== Phase 2: scores / softmax / outputs ==============
        with tc.tile_pool(name="p2sb", bufs=2) as p2sb, \
             tc.tile_pool(name="p2ps", bufs=2, space="PSUM") as p2ps, \
             tc.tile_pool(name="p2ps1", bufs=1, space="PSUM") as p2ps1:
            pvtp = p2ps1.tile([128, KB * H], FP)
            plp = p2ps1.tile([1, CS], FP)
            for h in range(H):
                scp = p2ps.tile([Q, K], FP, tag="scp", bufs=kb_["scpbufs"])
                for half in range(NH):
                    sl = slice(half * 512, half * 512 + SW)
                    nc.tensor.matmul(scp[:, sl], QTH[h][:], KTH[h][:, sl],
                                     start=True, stop=True)
                nc.vector.tensor_tensor(scp[:], scp[:], BFULL[:, :, h], OP.add)
                eh = p2sb.tile([Q, K], BF, tag="eh")
                zs = p2sb.tile([Q, 1], FP, tag="zs")
                nc.scalar.activation(eh[:], scp[:], AF.Exp, accum_out=zs[:])
                rz = p2sb.tile([Q, 1], FP, tag="rz")
                nc.vector.reciprocal(rz[:], zs[:])
                nc.scalar.mul(OUT[:, :, h], eh[:], rz[:])
                pz = p2sb.tile([Q, 1], BF, tag="pz")
                nc.vector.tensor_tensor(pz[:], PROT[:], rz[:], OP.mult)
                pvp = p2ps.tile([1, K], FP, tag="pvp", bufs=1)
                for half in range(NH):
                    sl = slice(half * 512, half * 512 + SW)
                    nc.tensor.matmul(pvp[0:1, sl], pz[:], eh[:, sl],
                                     start=True, stop=True)
                pve = p2sb.tile([1, K], FP, tag="pve")
                nc.scalar.copy(pve[:], pvp[:])
                for kc in range(KB):
                    i = h * KB + kc
                    nc.tensor.matmul(pvtp[:, i:i + 1],
                                     pve[0:1, kc * 128:(kc + 1) * 128],
                                     idf[0:1, 0:1], is_transpose=True,
                                     start=(i == 0), stop=(i == H * KB - 1))
            nc.vector.tensor_copy(PVT[:], pvtp[:])
            for h in range(H):
                for kc in range(KB):
                    nc.tensor.matmul(plp[0:1, h * DH:(h + 1) * DH],
                                     PVT[:, h * KB + kc:h * KB + kc + 1],
                                     V8[:, kc, h * DH:(h + 1) * DH],
                                     start=(h == 0 and kc == 0),
                                     stop=(h == H - 1 and kc == KB - 1))
            plo = p2sb.tile([1, CS], FP)
            nc.scalar.copy(plo[:], plp[:])
            nc.sync.dma_start(pool_o.ap(), plo[:])
            nc.sync.dma_start(attn_o.ap(), OUT[:])

        persist.release()
        consts.release()

    nc.compile()
    return nc


def host_prep(inputs, lc=LC):
    """Build per-core input maps from the full inputs."""
    f32 = np.float32
    z = np.ascontiguousarray(np.asarray(inputs["z"], f32))
    dist = np.ascontiguousarray(np.asarray(inputs["dist_bins"], f32))
    mask = np.asarray(inputs["pd_mask"])
    pc = np.asarray(inputs["prior_contact"], f32)
    pa = np.asarray(inputs["prior_pae"], f32)
    pd = np.asarray(inputs["prior_pde"], f32)
    sp = np.asarray(inputs["s_proxy"], f32)

    g = np.asarray(inputs["ln_z_g"], f32)
    b = np.asarray(inputs["ln_z_b"], f32)
    w_zb = np.asarray(inputs["w_zb"], f32)
    w8g = np.ascontiguousarray((g[:, None] * w_zb).astype(BF16))
    A2 = (g[:, None] * w_zb).sum(axis=0) * 0.5            # A_h / 2
    a2rep = np.ascontiguousarray(np.broadcast_to(A2.astype(f32), (128, H)))
    Bz = (b[:, None] * w_zb).sum(axis=0).astype(f32)      # [H]

    wd = np.asarray(inputs["w_dist"], f32)
    w2b = np.zeros((CP, 2 * H), f32)
    w2b[:BB, :H] = wd
    w2b[BB:, H:] = wd
    w2b = w2b.astype(BF16)

    sc = f32(1.0) / np.sqrt(f32(DH))

    def ext(w, bias, scale=f32(1.0)):
        e = np.zeros((CS + 1, CS), f32)
        e[:CS] = np.asarray(w, f32) * scale
        e[CS] = np.asarray(bias, f32) * scale
        return np.ascontiguousarray(e.astype(BF16))

    wqe = ext(inputs["w_q"], inputs["b_q"], sc)
    wke = ext(inputs["w_k"], inputs["b_k"])
    wve = ext(inputs["w_v"], inputs["b_v"])

    Qr = lc // NCORES
    bzq = np.ascontiguousarray(np.broadcast_to(Bz[:, None].astype(BF16), (H, Qr)))
    onesk = np.ones((1, lc), BF16)
    gbs = np.ascontiguousarray(np.stack(
        [np.asarray(inputs["ln_s_g"], f32), np.asarray(inputs["ln_s_b"], f32)],
        axis=1))

    maps = []
    for c in range(NCORES):
        sl = slice(c * Qr, (c + 1) * Qr)
        maps.append({
            "z_s": z[sl], "d_s": dist[sl],
            "pc_s": pc[sl], "pa_s": pa[sl], "pd_s": pd[sl],
            "mk_s": mask[sl].astype(np.uint8),
            "sprox": sp, "sown": np.ascontiguousarray(sp[sl]),
            "wqe": wqe, "wke": wke, "wve": wve,
            "w8g": w8g, "w2b": w2b, "a2rep": a2rep, "bzq": bzq,
            "onesk": onesk, "gbs": gbs,
        })
    return maps


def host_finish(results, inputs):
    """Gather per-core results; tiny output MLP on the host (f32)."""
    f32 = np.float32
    attn = np.concatenate([np.asarray(r["attn_o"], f32) for r in results], axis=0)
    PP = np.sum([np.asarray(r["pool_o"][0], f32) for r in results],
                axis=0, dtype=f32)

    mask = np.asarray(inputs["pd_mask"])
    sp_ = f32(mask.any(axis=1).sum())
    cnt = np.maximum(sp_, f32(1.0))
    w_o = np.asarray(inputs["w_o"], f32)
    b_o = np.asarray(inputs["b_o"], f32)
    pooled = (PP @ w_o + sp_ * b_o) / cnt
    m = pooled.mean(dtype=f32)
    v = f32(np.mean(np.square(pooled - m), dtype=f32))
    hn = (pooled - m) / np.sqrt(v + f32(1e-5))
    hn = hn * np.asarray(inputs["mlp_ln_g"], f32) + np.asarray(inputs["mlp_ln_b"], f32)
    a = np.maximum(hn @ np.asarray(inputs["mlp_w1"], f32)
                   + np.asarray(inputs["mlp_b1"], f32), f32(0.0))
    aff = (a @ np.asarray(inputs["mlp_w2"], f32)
           + np.asarray(inputs["mlp_b2"], f32)).reshape(())
    return np.asarray(aff, f32), attn


_NC_CACHE = {}


def _get_nc(lc=LC):
    if lc not in _NC_CACHE:
        _NC_CACHE[lc] = build_program(lc)
    return _NC_CACHE[lc]


def kernel(**inputs):
    nc = _get_nc()
    in_maps = host_prep(inputs)
    res = run_bass_kernel_spmd(nc, in_maps, core_ids=list(range(NCORES)))
    return host_finish(res.results, inputs)
